# revision 10
# baseline (speedup 1.0000x reference)
"""Trainium2 Bass kernel for 16-head causal attention prefill (B=2, T=2048, D=2048).

Sharding: 8 cores = 2 batches x 4 head-groups; core c handles batch c//4,
heads 4*(c%4) .. 4*(c%4)+3 (tensor-parallel over heads within a batch).
Each core computes its heads' QKV projection, causal attention, and the
partial output projection over its 512 feature columns; the host sums the
4 partial projections per batch (the "all-reduce" of the TP scheme) and
adds the projection bias.

All matmuls run as float32r (FP22 truncated fp32) on the PE at full rate.
Returns (out, k, v) matching the reference prefill module.
"""

import math

import numpy as np

import concourse.bass as bass  # noqa: F401  (engine types via nc)
import concourse.mybir as mybir
import concourse.tile as tile
from concourse import bacc
from concourse.bass_utils import run_bass_kernel_spmd

F32 = mybir.dt.float32
F32R = mybir.dt.float32r
EXP = mybir.ActivationFunctionType.Exp

B, T, D = 2, 2048, 2048
H = 16
DH = 128  # head dim
HPC = 4  # heads per core
NCORES = 8
KT = D // 128  # 16 contraction tiles
TSB = T // 512  # 4 token superblocks
TB = T // 128  # 16 token blocks
NEG = -30000.0

_CACHE = {}


def _build():
    nc = bacc.Bacc("TRN2", target_bir_lowering=False, debug=False, num_devices=NCORES)

    xt_ext = nc.dram_tensor("xt", [KT, 128, T], F32, kind="ExternalInput")
    wqk_ext = nc.dram_tensor("wqk", [8, KT, 128, 128], F32, kind="ExternalInput")
    wv_ext = nc.dram_tensor("wv", [KT, 128, 512], F32, kind="ExternalInput")
    bqk_ext = nc.dram_tensor("bqk", [128, 8], F32, kind="ExternalInput")
    bvb_ext = nc.dram_tensor("bvb", [128, 512], F32, kind="ExternalInput")
    wp_ext = nc.dram_tensor("wp", [HPC, 128, D], F32, kind="ExternalInput")
    mask_ext = nc.dram_tensor("mask", [128, 896], F32, kind="ExternalInput")
    ones_ext = nc.dram_tensor("ones", [128, 1], F32, kind="ExternalInput")
    onesr_ext = nc.dram_tensor("ones_row", [1, 128], F32, kind="ExternalInput")

    kt_out_ext = nc.dram_tensor("kt_out", [HPC, 128, T], F32, kind="ExternalOutput")
    v_out_ext = nc.dram_tensor("v_out", [TB, 128, 512], F32, kind="ExternalOutput")
    y_out_ext = nc.dram_tensor("y_out", [TB, 128, T], F32, kind="ExternalOutput")

    with tile.TileContext(nc) as tc:
        with (
            tc.tile_pool(name="const", bufs=1) as constp,
            tc.tile_pool(name="dram", bufs=1, space="DRAM") as dramp,
        ):
            mask_t = constp.tile([128, 896], F32)
            nc.sync.dma_start(mask_t[:], mask_ext[:])
            ones_t = constp.tile([128, 1], F32R)
            nc.sync.dma_start(ones_t[:], ones_ext[:].bitcast(F32R))
            onesr_t = constp.tile([1, 128], F32R)
            nc.sync.dma_start(onesr_t[:], onesr_ext[:].bitcast(F32R))
            bqk_t = constp.tile([128, 8], F32)
            nc.sync.dma_start(bqk_t[:], bqk_ext[:])
            bvb_t = constp.tile([128, 512], F32)
            nc.sync.dma_start(bvb_t[:], bvb_ext[:])

            qk_spill = dramp.tile([8, 128, T], F32)
            v_spill = dramp.tile([HPC, TB, 128, 128], F32)  # head-major

            # ---------------- Phase 1: QKV projection ----------------
            with tc.tile_pool(name="xt", bufs=1) as xtp:
                xt_t = xtp.tile([128, KT, T], F32R)

                # v first (in natural [t, f] layout, f = 4 heads x 128) so that
                # v_spill is ready early and wv's SBUF zone frees early for
                # phase-2 pools. wv/xt loads interleaved so the first v matmul
                # chain starts after ~2MB of DMA instead of ~20MB.
                with (
                    tc.tile_pool(name="wv", bufs=1) as wvp,
                    tc.tile_pool(name="st1b", bufs=4) as st1bp,
                    tc.tile_pool(name="psv", bufs=4, space="PSUM") as psvp,
                ):
                    wv_t = wvp.tile([128, KT, 512], F32R)
                    for kt in range(KT):
                        nc.sync.dma_start(wv_t[:, kt, :], wv_ext[kt].bitcast(F32R))
                        nc.sync.dma_start(xt_t[:, kt, :], xt_ext[kt].bitcast(F32R))
                    for tb in range(TB):
                        psv = psvp.tile([128, 512], F32, tag="psv")
                        for kt in range(KT):
                            nc.tensor.matmul(
                                psv[:],
                                xt_t[:, kt, 128 * tb : 128 * (tb + 1)],
                                wv_t[:, kt, :],
                                start=(kt == 0),
                                stop=(kt == KT - 1),
                            )
                        stgv = st1bp.tile([128, 512], F32, tag="stg")
                        nc.vector.tensor_add(stgv[:], psv[:], bvb_t[:])
                        for j in range(HPC):
                            nc.sync.dma_start(
                                v_spill[j, tb], stgv[:, 128 * j : 128 * (j + 1)]
                            )
                        nc.sync.dma_start(v_out_ext[tb], stgv[:])

                # q (fb 0-3, scaled) and k (fb 4-7): qT/kT[f, t] layout.
                # Interleave q/k per head so early heads' spills complete early.
                with (
                    tc.tile_pool(name="w1", bufs=2) as w1p,
                    tc.tile_pool(name="st1a", bufs=4) as st1p,
                    tc.tile_pool(name="ps1", bufs=2, space="PSUM") as ps1p,
                ):
                    for fb in (0, 4, 1, 5, 2, 6, 3, 7):
                        w_t = w1p.tile([128, KT, 128], F32R, tag="w1")
                        for kt in range(KT):
                            nc.sync.dma_start(
                                w_t[:, kt, :], wqk_ext[fb, kt].bitcast(F32R)
                            )
                        ps_t = ps1p.tile([128, TSB, 512], F32, tag="ps1")
                        for kt in range(KT):
                            for tsb in range(TSB):
                                nc.tensor.matmul(
                                    ps_t[:, tsb, :],
                                    w_t[:, kt, :],
                                    xt_t[:, kt, 512 * tsb : 512 * (tsb + 1)],
                                    start=(kt == 0),
                                    stop=(kt == KT - 1),
                                )
                        for tsb in range(TSB):
                            stg = st1p.tile([128, 512], F32, tag="stg")
                            nc.vector.tensor_scalar_add(
                                stg[:], ps_t[:, tsb, :], bqk_t[:, fb : fb + 1]
                            )
                            sl = slice(512 * tsb, 512 * (tsb + 1))
                            nc.sync.dma_start(qk_spill[fb, :, sl], stg[:])
                            if fb >= 4:
                                nc.sync.dma_start(kt_out_ext[fb - 4, :, sl], stg[:])

            # -------- Phase 2 + 3: causal attention + output projection ------
            outup = tc.alloc_tile_pool(name="outu", bufs=1)
            # outU[d, 4*h + tsb, t]: normalized attention output, f32r for proj
            outU = outup.tile([128, 16, 512], F32R)
            with (
                tc.tile_pool(name="qk2", bufs=2) as qk2p,
                tc.tile_pool(name="v2", bufs=2) as v2p,
                tc.tile_pool(name="pt", bufs=4) as ptp,
                tc.tile_pool(name="ssb", bufs=2) as ssbp,
                tc.tile_pool(name="dn", bufs=2) as dnp,
                tc.tile_pool(name="wp", bufs=1) as wpp,
                tc.tile_pool(name="st3", bufs=4) as st3p,
                tc.tile_pool(name="ps_s", bufs=3, space="PSUM") as pssp,
                tc.tile_pool(name="ps_o", bufs=2, space="PSUM") as psop,
                tc.tile_pool(name="ps_d", bufs=1, space="PSUM") as psdp,
                tc.tile_pool(name="ps3", bufs=2, space="PSUM") as ps3p,
            ):
                wp_t = wpp.tile([128, HPC, D], F32R)
                for kt in range(HPC):
                    nc.sync.dma_start(wp_t[:, kt, :], wp_ext[kt].bitcast(F32R))

                for h in range(HPC):
                    q_t = qk2p.tile([128, T], F32R, tag="q")
                    nc.sync.dma_start(q_t[:], qk_spill[h].bitcast(F32R))
                    k_t = qk2p.tile([128, T], F32R, tag="k")
                    nc.sync.dma_start(k_t[:], qk_spill[4 + h].bitcast(F32R))
                    v_t = v2p.tile([128, TB, 128], F32R, tag="v")
                    for tb in range(TB):
                        nc.sync.dma_start(
                            v_t[:, tb, :], v_spill[h, tb].bitcast(F32R)
                        )

                    for tsb in range(TSB):
                        nt = 4 * (tsb + 1)  # causal tk blocks
                        ps_o = psop.tile([128, 512], F32, tag="ps_o")
                        ps_d = psdp.tile([1, 512], F32, tag="ps_d")
                        qsl = q_t[:, 512 * tsb : 512 * (tsb + 1)]
                        for tkb in range(nt):
                            ps_s = pssp.tile([128, 512], F32, tag="ps_s")
                            nc.tensor.matmul(
                                ps_s[:],
                                k_t[:, 128 * tkb : 128 * (tkb + 1)],
                                qsl,
                                start=True,
                                stop=True,
                            )
                            pt_t = ptp.tile([128, 512], F32R, tag="pt")
                            if tkb >= 4 * tsb:
                                i = tkb - 4 * tsb
                                s_sb = ssbp.tile([128, 512], F32, tag="ssb")
                                nc.vector.tensor_add(
                                    s_sb[:],
                                    ps_s[:],
                                    mask_t[:, 384 - 128 * i : 896 - 128 * i],
                                )
                                nc.scalar.activation(pt_t[:], s_sb[:], EXP)
                            else:
                                nc.scalar.activation(pt_t[:], ps_s[:], EXP)
                            nc.tensor.matmul(
                                ps_o[:],
                                v_t[:, tkb, :],
                                pt_t[:],
                                start=(tkb == 0),
                                stop=(tkb == nt - 1),
                            )
                            nc.tensor.matmul(
                                ps_d[:],
                                ones_t[:],
                                pt_t[:],
                                start=(tkb == 0),
                                stop=(tkb == nt - 1),
                            )
                        # denominator -> broadcast -> reciprocal -> normalize
                        d_sb = dnp.tile([1, 512], F32, tag="dsb")
                        nc.vector.tensor_copy(d_sb[:], ps_d[:])
                        D_t = dnp.tile([128, 512], F32, tag="Dt")
                        nc.gpsimd.partition_broadcast(D_t[:], d_sb[:])
                        r_t = dnp.tile([128, 512], F32, tag="rt")
                        nc.vector.reciprocal_approx_fast(r_t[:], D_t[:])
                        nc.vector.tensor_mul(
                            outU[:, 4 * h + tsb, :], ps_o[:], r_t[:]
                        )

                # Phase 3 (emitted last; overlaps attention tail per tsb since
                # proj for tsb only needs outU[:, 4h+tsb, :] of all heads)
                for tsb in range(TSB):
                    for db in range(TB):
                        ps_y = ps3p.tile([128, 512], F32, tag="ps3")
                        for kt in range(HPC):
                            nc.tensor.matmul(
                                ps_y[:],
                                wp_t[:, kt, 128 * db : 128 * (db + 1)],
                                outU[:, 4 * kt + tsb, :],
                                start=(kt == 0),
                                stop=(kt == HPC - 1),
                            )
                        stg = st3p.tile([128, 512], F32, tag="st3")
                        nc.vector.tensor_copy(stg[:], ps_y[:])
                        nc.sync.dma_start(
                            y_out_ext[db, :, 512 * tsb : 512 * (tsb + 1)], stg[:]
                        )
            outup.release()

    nc.compile()
    return nc


def _prep_in_maps(inputs, w_qkv, b_qkv, w_proj):
    scale = 1.0 / math.sqrt(DH)
    mask = np.full((128, 896), NEG, np.float32)
    p_idx = np.arange(128)[:, None]
    c_idx = np.arange(896)[None, :]
    mask[p_idx <= c_idx - 384] = 0.0
    ones = np.ones((128, 1), np.float32)
    ones_row = np.ones((1, 128), np.float32)

    wqkT = np.ascontiguousarray(w_qkv.T)  # [D, 3D]
    wpT = np.ascontiguousarray(w_proj.T)  # [D, D]

    in_maps = []
    for c in range(NCORES):
        b = c // 4
        hg = c % 4
        heads = [4 * hg + j for j in range(HPC)]

        xt = np.ascontiguousarray(inputs[b].T).reshape(KT, 128, T)

        wqk = np.empty((8, KT, 128, 128), np.float32)
        bqk = np.empty((128, 8), np.float32)
        wv = np.empty((KT, 128, HPC, 128), np.float32)
        bvb = np.empty((128, HPC, 128), np.float32)
        for j, h in enumerate(heads):
            r0 = h * 3 * DH
            wqk[j] = (wqkT[:, r0 : r0 + DH] * scale).reshape(KT, 128, 128)
            wqk[4 + j] = wqkT[:, r0 + DH : r0 + 2 * DH].reshape(KT, 128, 128)
            bqk[:, j] = b_qkv[r0 : r0 + DH] * scale
            bqk[:, 4 + j] = b_qkv[r0 + DH : r0 + 2 * DH]
            wv[:, :, j, :] = wqkT[:, r0 + 2 * DH : r0 + 3 * DH].reshape(KT, 128, 128)
            bvb[:, j, :] = b_qkv[r0 + 2 * DH : r0 + 3 * DH][None, :]
        wv = wv.reshape(KT, 128, 512)
        bvb = bvb.reshape(128, 512)

        d0 = 4 * hg * DH
        wp = np.ascontiguousarray(wpT[d0 : d0 + 512, :]).reshape(HPC, 128, D)

        in_maps.append(
            {
                "xt": np.ascontiguousarray(xt),
                "wqk": np.ascontiguousarray(wqk),
                "wv": np.ascontiguousarray(wv),
                "bqk": np.ascontiguousarray(bqk),
                "bvb": np.ascontiguousarray(bvb),
                "wp": wp,
                "mask": mask,
                "ones": ones,
                "ones_row": ones_row,
            }
        )
    return in_maps


def kernel_run(inputs, w_qkv, b_qkv, w_proj, b_proj, trace=False):
    """Run the kernel; returns ((out, k, v), exec_time_ns)."""
    inputs = np.asarray(inputs, np.float32)
    w_qkv = np.asarray(w_qkv, np.float32)
    b_qkv = np.asarray(b_qkv, np.float32)
    w_proj = np.asarray(w_proj, np.float32)
    b_proj = np.asarray(b_proj, np.float32)

    if "nc" not in _CACHE:
        _CACHE["nc"] = _build()
    nc = _CACHE["nc"]

    in_maps = _prep_in_maps(inputs, w_qkv, b_qkv, w_proj)
    res = run_bass_kernel_spmd(
        nc, in_maps, core_ids=list(range(NCORES)), trace=trace
    )
    outs = res.results

    k = np.empty((B, H, T, DH), np.float32)
    v = np.empty((B, H, T, DH), np.float32)
    y = np.zeros((B, T, D), np.float32)
    for c in range(NCORES):
        b = c // 4
        hg = c % 4
        kt_o = outs[c]["kt_out"]  # [4, 128, T]
        v_o = outs[c]["v_out"].reshape(T, 512)  # [T, 4*128]
        for j in range(HPC):
            k[b, 4 * hg + j] = kt_o[j].T
            v[b, 4 * hg + j] = v_o[:, 128 * j : 128 * (j + 1)]
        y[b] += outs[c]["y_out"].reshape(D, T).T
    y += b_proj[None, None, :]
    return (y, k, v), res.exec_time_ns


def kernel(inputs, w_qkv, b_qkv, w_proj, b_proj):
    out, _ = kernel_run(inputs, w_qkv, b_qkv, w_proj, b_proj, trace=False)
    return out


# revision 14
# speedup vs baseline: 1.0745x; 1.0745x over previous
"""Trainium2 Bass kernel for 16-head causal attention prefill (B=2, T=2048, D=2048).

Sharding: 8 cores = 2 batches x 4 head-groups; core c handles batch c//4,
heads 4*(c%4) .. 4*(c%4)+3 (tensor-parallel over heads within a batch).
Each core computes its heads' QKV projection, causal attention, and the
partial output projection over its 512 feature columns; the host sums the
4 partial projections per batch (the "all-reduce" of the TP scheme) and
adds the projection bias.

QKV projection + attention run in bf16 on the PE (1 cyc/row, FWL weight
loads); the output projection runs in float32r. PSUM accumulation is fp32
throughout. k/v/y ship back as bf16; the host assembles fp32 outputs.
Returns (out, k, v) matching the reference prefill module.
"""

import math

import numpy as np
import ml_dtypes

import concourse.bass as bass  # noqa: F401
import concourse.mybir as mybir
import concourse.tile as tile
from concourse import bacc
from concourse.bass_utils import run_bass_kernel_spmd

F32 = mybir.dt.float32
F32R = mybir.dt.float32r
BF16 = mybir.dt.bfloat16
EXP = mybir.ActivationFunctionType.Exp
BF16NP = ml_dtypes.bfloat16

B, T, D = 2, 2048, 2048
H = 16
DH = 128  # head dim
HPC = 4  # heads per core
NCORES = 8
KT = D // 128  # 16 contraction tiles
TSB = T // 512  # 4 token superblocks
TB = T // 128  # 16 token blocks
NEG = -30000.0

_CACHE = {}


def _build():
    nc = bacc.Bacc("TRN2", target_bir_lowering=False, debug=False, num_devices=NCORES)

    xt_ext = nc.dram_tensor("xt", [KT, 128, T], BF16, kind="ExternalInput")
    wqk_ext = nc.dram_tensor("wqk", [8, KT, 128, 128], BF16, kind="ExternalInput")
    wv_ext = nc.dram_tensor("wv", [KT, 128, 512], BF16, kind="ExternalInput")
    bqk_ext = nc.dram_tensor("bqk", [128, 8], F32, kind="ExternalInput")
    bvb_ext = nc.dram_tensor("bvb", [128, 512], F32, kind="ExternalInput")
    wp_ext = nc.dram_tensor("wp", [HPC, 128, D], F32, kind="ExternalInput")
    mask_ext = nc.dram_tensor("mask", [128, 896], F32, kind="ExternalInput")
    ones_ext = nc.dram_tensor("ones", [128, 1], BF16, kind="ExternalInput")

    kt_out_ext = nc.dram_tensor("kt_out", [HPC, 128, T], BF16, kind="ExternalOutput")
    v_out_ext = nc.dram_tensor("v_out", [TB, 128, 512], BF16, kind="ExternalOutput")
    y_out_ext = nc.dram_tensor("y_out", [TB, 128, T], BF16, kind="ExternalOutput")

    with tile.TileContext(nc) as tc:
        with (
            tc.tile_pool(name="const", bufs=1) as constp,
            tc.tile_pool(name="dram", bufs=1, space="DRAM") as dramp,
        ):
            mask_t = constp.tile([128, 896], F32)
            nc.sync.dma_start(mask_t[:], mask_ext[:])
            ones_t = constp.tile([128, 1], BF16)
            nc.sync.dma_start(ones_t[:], ones_ext[:])
            bqk_t = constp.tile([128, 8], F32)
            nc.sync.dma_start(bqk_t[:], bqk_ext[:])
            bvb_t = constp.tile([128, 512], F32)
            nc.sync.dma_start(bvb_t[:], bvb_ext[:])

            qk_spill = dramp.tile([8, 128, T], BF16)
            v_spill = dramp.tile([HPC, TB, 128, 128], BF16)  # head-major

            # Pre-allocate phase-2 input pools + phase-1 qk pools so their
            # SBUF zones don't overlap phase-1's xt/wv zones: their DMAs can
            # then overlap the phase-1 tail instead of waiting on pool
            # release barriers.
            qk2p = tc.alloc_tile_pool(name="qk2", bufs=2)
            v2p = tc.alloc_tile_pool(name="v2", bufs=2)
            wpp = tc.alloc_tile_pool(name="wp", bufs=1)
            w1p = tc.alloc_tile_pool(name="w1", bufs=2)
            st1p = tc.alloc_tile_pool(name="st1a", bufs=4)
            ps1p = tc.alloc_tile_pool(name="ps1", bufs=1, space="PSUM")

            # ---------------- Phase 1: QKV projection ----------------
            with tc.tile_pool(name="xt", bufs=1) as xtp:
                xt_t = xtp.tile([128, KT, T], BF16)

                # v first (natural [t, f] layout, f = 4 heads x 128): v_spill
                # ready early, wv zone frees early. wv/xt loads interleaved so
                # the first v matmul chain starts after ~1MB of DMA.
                with (
                    tc.tile_pool(name="wv", bufs=1) as wvp,
                    tc.tile_pool(name="st1b", bufs=4) as st1bp,
                    tc.tile_pool(name="psv", bufs=4, space="PSUM") as psvp,
                ):
                    wv_t = wvp.tile([128, KT, 512], BF16)
                    for kt in range(KT):
                        nc.sync.dma_start(wv_t[:, kt, :], wv_ext[kt])
                        nc.sync.dma_start(xt_t[:, kt, :], xt_ext[kt])
                    for tb in range(TB):
                        psv = psvp.tile([128, 512], F32, tag="psv")
                        for kt in range(KT):
                            nc.tensor.matmul(
                                psv[:],
                                xt_t[:, kt, 128 * tb : 128 * (tb + 1)],
                                wv_t[:, kt, :],
                                start=(kt == 0),
                                stop=(kt == KT - 1),
                            )
                        stgv = st1bp.tile([128, 512], BF16, tag="stg")
                        nc.vector.tensor_add(stgv[:], psv[:], bvb_t[:])
                        for j in range(HPC):
                            nc.sync.dma_start(
                                v_spill[j, tb], stgv[:, 128 * j : 128 * (j + 1)]
                            )
                        nc.sync.dma_start(v_out_ext[tb], stgv[:])

                # q (fb 0-3, scaled) and k (fb 4-7): qT/kT[f, t] layout.
                # Interleave q/k per head so early heads' spills complete early.
                for fb in (0, 4, 1, 5, 2, 6, 3, 7):
                    w_t = w1p.tile([128, KT, 128], BF16, tag="w1")
                    for kt in range(KT):
                        nc.sync.dma_start(w_t[:, kt, :], wqk_ext[fb, kt])
                    ps_t = ps1p.tile([128, TSB, 512], F32, tag="ps1")
                    for kt in range(KT):
                        for tsb in range(TSB):
                            nc.tensor.matmul(
                                ps_t[:, tsb, :],
                                w_t[:, kt, :],
                                xt_t[:, kt, 512 * tsb : 512 * (tsb + 1)],
                                start=(kt == 0),
                                stop=(kt == KT - 1),
                            )
                    for tsb in range(TSB):
                        stg = st1p.tile([128, 512], BF16, tag="stg")
                        nc.vector.tensor_scalar_add(
                            stg[:], ps_t[:, tsb, :], bqk_t[:, fb : fb + 1]
                        )
                        sl = slice(512 * tsb, 512 * (tsb + 1))
                        nc.sync.dma_start(qk_spill[fb, :, sl], stg[:])
                        if fb >= 4:
                            nc.sync.dma_start(kt_out_ext[fb - 4, :, sl], stg[:])

            ps1p.release()
            st1p.release()
            w1p.release()

            # -------- Phase 2 + 3: causal attention + output projection ------
            outup = tc.alloc_tile_pool(name="outu", bufs=1)
            # outU[d, 4*h + tsb, t]: normalized attention output, f32r for proj
            outU = outup.tile([128, 16, 512], F32R)
            with (
                tc.tile_pool(name="pt", bufs=6) as ptp,
                tc.tile_pool(name="ssb", bufs=2) as ssbp,
                tc.tile_pool(name="dn", bufs=2) as dnp,
                tc.tile_pool(name="st3", bufs=4) as st3p,
                tc.tile_pool(name="ps_s", bufs=3, space="PSUM") as pssp,
                tc.tile_pool(name="ps_o", bufs=2, space="PSUM") as psop,
                tc.tile_pool(name="ps_d", bufs=1, space="PSUM") as psdp,
                tc.tile_pool(name="ps3", bufs=2, space="PSUM") as ps3p,
            ):
                first_wp = True
                for h in range(HPC):
                    q_t = qk2p.tile([128, T], BF16, tag="q")
                    nc.sync.dma_start(q_t[:], qk_spill[h])
                    k_t = qk2p.tile([128, T], BF16, tag="k")
                    nc.sync.dma_start(k_t[:], qk_spill[4 + h])
                    v_t = v2p.tile([128, TB, 128], BF16, tag="v")
                    for tb in range(TB):
                        nc.sync.dma_start(v_t[:, tb, :], v_spill[h, tb])
                    if first_wp:
                        # emitted after h0 loads so it doesn't delay them
                        first_wp = False
                        wp_t = wpp.tile([128, HPC, D], F32R)
                        for kt in range(HPC):
                            nc.sync.dma_start(
                                wp_t[:, kt, :], wp_ext[kt].bitcast(F32R)
                            )

                    for tsb in range(TSB):
                        nt = 4 * (tsb + 1)  # causal tk blocks
                        ps_o = psop.tile([128, 512], F32, tag="ps_o")
                        ps_d = psdp.tile([1, 512], F32, tag="ps_d")
                        qsl = q_t[:, 512 * tsb : 512 * (tsb + 1)]
                        for tkb in range(nt):
                            ps_s = pssp.tile([128, 512], F32, tag="ps_s")
                            nc.tensor.matmul(
                                ps_s[:],
                                k_t[:, 128 * tkb : 128 * (tkb + 1)],
                                qsl,
                                start=True,
                                stop=True,
                            )
                            pt_t = ptp.tile([128, 512], BF16, tag="pt")
                            if tkb >= 4 * tsb:
                                i = tkb - 4 * tsb
                                s_sb = ssbp.tile([128, 512], F32, tag="ssb")
                                nc.vector.tensor_add(
                                    s_sb[:],
                                    ps_s[:],
                                    mask_t[:, 384 - 128 * i : 896 - 128 * i],
                                )
                                nc.scalar.activation(pt_t[:], s_sb[:], EXP)
                            else:
                                nc.scalar.activation(pt_t[:], ps_s[:], EXP)
                            nc.tensor.matmul(
                                ps_o[:],
                                v_t[:, tkb, :],
                                pt_t[:],
                                start=(tkb == 0),
                                stop=(tkb == nt - 1),
                            )
                            nc.tensor.matmul(
                                ps_d[:],
                                ones_t[:],
                                pt_t[:],
                                start=(tkb == 0),
                                stop=(tkb == nt - 1),
                            )
                        # denominator -> broadcast -> reciprocal -> normalize
                        d_sb = dnp.tile([1, 512], F32, tag="dsb")
                        nc.vector.tensor_copy(d_sb[:], ps_d[:])
                        D_t = dnp.tile([128, 512], F32, tag="Dt")
                        nc.gpsimd.partition_broadcast(D_t[:], d_sb[:])
                        r_t = dnp.tile([128, 512], F32, tag="rt")
                        nc.vector.reciprocal_approx_fast(r_t[:], D_t[:])
                        nc.vector.tensor_mul(
                            outU[:, 4 * h + tsb, :], ps_o[:], r_t[:]
                        )

                # Phase 3 (emitted last; overlaps attention tail per tsb since
                # proj for tsb only needs outU[:, 4h+tsb, :] of all heads)
                for tsb in range(TSB):
                    for db in range(TB):
                        ps_y = ps3p.tile([128, 512], F32, tag="ps3")
                        for kt in range(HPC):
                            nc.tensor.matmul(
                                ps_y[:],
                                wp_t[:, kt, 128 * db : 128 * (db + 1)],
                                outU[:, 4 * kt + tsb, :],
                                start=(kt == 0),
                                stop=(kt == HPC - 1),
                            )
                        stg = st3p.tile([128, 512], BF16, tag="st3")
                        nc.vector.tensor_copy(stg[:], ps_y[:])
                        nc.sync.dma_start(
                            y_out_ext[db, :, 512 * tsb : 512 * (tsb + 1)], stg[:]
                        )
            outup.release()
            wpp.release()
            v2p.release()
            qk2p.release()

    nc.compile()
    return nc


def _prep_in_maps(inputs, w_qkv, b_qkv, w_proj):
    scale = 1.0 / math.sqrt(DH)
    mask = np.full((128, 896), NEG, np.float32)
    p_idx = np.arange(128)[:, None]
    c_idx = np.arange(896)[None, :]
    mask[p_idx <= c_idx - 384] = 0.0
    ones = np.ones((128, 1), BF16NP)

    wqkT = np.ascontiguousarray(w_qkv.T)  # [D, 3D]
    wpT = np.ascontiguousarray(w_proj.T)  # [D, D]

    in_maps = []
    for c in range(NCORES):
        b = c // 4
        hg = c % 4
        heads = [4 * hg + j for j in range(HPC)]

        xt = np.ascontiguousarray(inputs[b].T).reshape(KT, 128, T)

        wqk = np.empty((8, KT, 128, 128), np.float32)
        bqk = np.empty((128, 8), np.float32)
        wv = np.empty((KT, 128, HPC, 128), np.float32)
        bvb = np.empty((128, HPC, 128), np.float32)
        for j, h in enumerate(heads):
            r0 = h * 3 * DH
            wqk[j] = (wqkT[:, r0 : r0 + DH] * scale).reshape(KT, 128, 128)
            wqk[4 + j] = wqkT[:, r0 + DH : r0 + 2 * DH].reshape(KT, 128, 128)
            bqk[:, j] = b_qkv[r0 : r0 + DH] * scale
            bqk[:, 4 + j] = b_qkv[r0 + DH : r0 + 2 * DH]
            wv[:, :, j, :] = wqkT[:, r0 + 2 * DH : r0 + 3 * DH].reshape(KT, 128, 128)
            bvb[:, j, :] = b_qkv[r0 + 2 * DH : r0 + 3 * DH][None, :]
        wv = wv.reshape(KT, 128, 512)
        bvb = bvb.reshape(128, 512)

        d0 = 4 * hg * DH
        wp = np.ascontiguousarray(wpT[d0 : d0 + 512, :]).reshape(HPC, 128, D)

        in_maps.append(
            {
                "xt": xt.astype(BF16NP),
                "wqk": wqk.astype(BF16NP),
                "wv": wv.astype(BF16NP),
                "bqk": np.ascontiguousarray(bqk),
                "bvb": np.ascontiguousarray(bvb),
                "wp": wp,
                "mask": mask,
                "ones": ones,
            }
        )
    return in_maps


def kernel_run(inputs, w_qkv, b_qkv, w_proj, b_proj, trace=False):
    """Run the kernel; returns ((out, k, v), exec_time_ns)."""
    inputs = np.asarray(inputs, np.float32)
    w_qkv = np.asarray(w_qkv, np.float32)
    b_qkv = np.asarray(b_qkv, np.float32)
    w_proj = np.asarray(w_proj, np.float32)
    b_proj = np.asarray(b_proj, np.float32)

    if "nc" not in _CACHE:
        _CACHE["nc"] = _build()
    nc = _CACHE["nc"]

    in_maps = _prep_in_maps(inputs, w_qkv, b_qkv, w_proj)
    res = run_bass_kernel_spmd(
        nc, in_maps, core_ids=list(range(NCORES)), trace=trace
    )
    outs = res.results

    k = np.empty((B, H, T, DH), np.float32)
    v = np.empty((B, H, T, DH), np.float32)
    y = np.zeros((B, T, D), np.float32)
    for c in range(NCORES):
        b = c // 4
        hg = c % 4
        kt_o = np.asarray(outs[c]["kt_out"], dtype=np.float32)  # [4, 128, T]
        v_o = np.asarray(outs[c]["v_out"], dtype=np.float32).reshape(T, 512)
        for j in range(HPC):
            k[b, 4 * hg + j] = kt_o[j].T
            v[b, 4 * hg + j] = v_o[:, 128 * j : 128 * (j + 1)]
        y[b] += np.asarray(outs[c]["y_out"], dtype=np.float32).reshape(D, T).T
    y += b_proj[None, None, :]
    return (y, k, v), res.exec_time_ns


def kernel(inputs, w_qkv, b_qkv, w_proj, b_proj):
    out, _ = kernel_run(inputs, w_qkv, b_qkv, w_proj, b_proj, trace=False)
    return out
